# revision 6
# baseline (speedup 1.0000x reference)
"""BertGCN fused kernel for 8x TRN2 NeuronCores.

Math (reference):
    X = label_features @ gc_weight                      # [L, H]
    E = relu(edges @ X + gc_bias)                       # [L, H]
    diag = sum(E * clf_weight, axis=1)                  # [L]
    out = bert_cls @ clf_weight.T + diag[None] + clf_bias[None]   # [B, L]

Sharding: label dim L split over 8 cores (1024 labels each). Each core:
  stage 1: X = LF @ GCW (full, bf16)               [8192, 1024]
  stage 2: E_c = relu(edges_c @ X + gc_bias)       [1024, 1024]
  diag_c  = rowsum(E_c * W_c)                      [1024]
  stage 3: out_c.T = W_c @ bert.T + diag_c + bias  [1024, 2048]  (fp16 matmul)
Host pre-transposes/tiles/casts operands (layout only, no FLOPs) and
re-assembles out = vstack(out_c.T).T.

B, H, L, F = 2048, 1024, 8192, 1024.
"""

import os
import numpy as np
import ml_dtypes

STAGES = os.environ.get("KSTAGES", "123")
NOBIAS = os.environ.get("KNOBIAS", "0") == "1"
NODIAG = os.environ.get("KNODIAG", "0") == "1"

B, H, L, F = 2048, 1024, 8192, 1024
NCORES = 8
LS = L // NCORES  # 1024 labels per core
P = 128

LAST_RESULT = None


def build_kernel():
    import concourse.bass as bass  # noqa: F401
    from concourse import bacc
    import concourse.mybir as mybir
    import concourse.tile as tile

    dt = mybir.dt
    f32, bf16, f16 = dt.float32, dt.bfloat16, dt.float16
    Relu = mybir.ActivationFunctionType.Relu
    mult, add = mybir.AluOpType.mult, mybir.AluOpType.add

    nc = bacc.Bacc(None, target_bir_lowering=False, debug=False)

    # inputs (per core; layouts chosen so every DMA is contiguous-ish)
    lf = nc.declare_dram_parameter("lf_tiles", [8, 64, P, P], bf16, isOutput=False)
    gcw = nc.declare_dram_parameter("gcw", [F, H], bf16, isOutput=False)
    gcb = nc.declare_dram_parameter("gcb_row", [1, H], bf16, isOutput=False)
    edg = nc.declare_dram_parameter("edges_tiles", [8, 64, P, P], bf16, isOutput=False)
    cwt = nc.declare_dram_parameter("clfwt_tiles", [8, 8, P, P], f16, isOutput=False)
    brt = nc.declare_dram_parameter("bert_t", [H, B], f16, isOutput=False)
    cw = nc.declare_dram_parameter("clfw", [LS, H], f32, isOutput=False)
    cb = nc.declare_dram_parameter("clfb_col", [LS, 1], f32, isOutput=False)
    out = nc.declare_dram_parameter("out_t", [LS, B], f32, isOutput=True)

    KX = F // P      # 8   k-chunks for stage 1 (over F)
    NLP = L // P     # 64  l'-blocks (stage-1 M / stage-2 K)
    KL = 8192 // P   # 64  stage-2 k-chunks (over l')
    NLB = LS // P    # 8   l-blocks of this core's label shard
    NH2 = H // 512   # 2   h-halves
    NB4 = B // 512   # 4   b-quarters (stage 3 N)
    KH = H // P      # 8   stage-3 k-chunks (over H)

    with tile.TileContext(nc) as tc:
        with (
            tc.tile_pool(name="const", bufs=1) as constp,
            tc.tile_pool(name="xk", bufs=NLP) as xpool,
            tc.tile_pool(name="wstream", bufs=16) as wpool,
            tc.tile_pool(name="bstream", bufs=2) as bpool,
            tc.tile_pool(name="estream", bufs=2) as epool,
            tc.tile_pool(name="cwstream", bufs=2) as cwpool,
            tc.tile_pool(name="opool", bufs=2) as opool,
            tc.tile_pool(name="psx", bufs=2, space="PSUM") as psx,
            tc.tile_pool(name="pse", bufs=2, space="PSUM") as pse,
            tc.tile_pool(name="pso", bufs=4, space="PSUM") as pso,
        ):
            # ---- resident constants ----
            gcw_sb = constp.tile([P, KX, H], bf16, tag="gcw")  # [f_in, f_chunk, h]
            nc.sync.dma_start(
                out=gcw_sb[:], in_=gcw.rearrange("(k p) h -> p k h", p=P)
            )
            gcb_sb = constp.tile([1, H], bf16, tag="gcb")
            nc.sync.dma_start(out=gcb_sb[:], in_=gcb[:])
            ones1 = constp.tile([1, P], bf16, tag="ones1")
            nc.vector.memset(ones1[:], 1.0)
            cwt_sb = constp.tile([P, NLB, KH, P], f16, tag="cwt")  # [h_in, lb, h_chunk, l_in]
            for lb in range(NLB):
                for k in range(KH):
                    nc.sync.dma_start(out=cwt_sb[:, lb, k, :], in_=cwt[lb, k])
            bias_cols = constp.tile([P, NLB], f32, tag="bias")  # diag+clf_bias per l-block
            dscratch = constp.tile([P, H], f32, tag="dscratch")
            dcol = constp.tile([P, 1], f32, tag="dcol")

            # ---- stage 1: X[l', h] = LF @ GCW  (bf16) ----
            x_sb = []
            for j in range(NLP):
                x_sb.append(xpool.tile([P, H], bf16, tag="xk", name=f"x{j}"))
            if "1" not in STAGES:
                for j in range(NLP):
                    nc.vector.memset(x_sb[j][:], 0.0)
            if "2" not in STAGES or NODIAG:
                nc.vector.memset(bias_cols[:], 0.0)
            for j in (range(NLP) if "1" in STAGES else []):
                ps = [psx.tile([P, 512], f32, tag="psx", name=f"psx{j}_{h}") for h in range(NH2)]
                for k in range(KX):
                    w = wpool.tile([P, P], bf16, tag="w")
                    nc.sync.dma_start(out=w[:], in_=lf[k, j])
                    for h in range(NH2):
                        nc.tensor.matmul(
                            ps[h][:],
                            w[:],
                            gcw_sb[:, k, 512 * h : 512 * (h + 1)],
                            start=(k == 0),
                            stop=(k == KX - 1),
                        )
                for h in range(NH2):
                    nc.scalar.copy(x_sb[j][:, 512 * h : 512 * (h + 1)], ps[h][:])

            # ---- stage 2: E = relu(edges_c @ X + gc_bias); diag ----
            for lb in (range(NLB) if "2" in STAGES else []):
                e_sb = epool.tile([P, H], f32, tag="e")
                ps = [pse.tile([P, 512], f32, tag="pse", name=f"pse{lb}_{h}") for h in range(NH2)]
                for k in range(KL):
                    w = wpool.tile([P, P], bf16, tag="w")
                    nc.sync.dma_start(out=w[:], in_=edg[lb, k])
                    for h in range(NH2):
                        nc.tensor.matmul(
                            ps[h][:],
                            w[:],
                            x_sb[k][:, 512 * h : 512 * (h + 1)],
                            start=(k == 0),
                            stop=(NOBIAS and k == KL - 1),
                        )
                for h in range(NH2):
                    # + gc_bias via K=1 accumulation row
                    if not NOBIAS:
                        nc.tensor.matmul(
                            ps[h][:],
                            ones1[:],
                            gcb_sb[:, 512 * h : 512 * (h + 1)],
                            start=False,
                            stop=True,
                        )
                    nc.scalar.activation(
                        e_sb[:, 512 * h : 512 * (h + 1)], ps[h][:], Relu
                    )
                # diag_c[lb] = rowsum(E_lb * W_lb), fused mul+reduce on DVE
                if NODIAG:
                    continue
                cw_sb = cwpool.tile([P, H], f32, tag="cw")
                nc.sync.dma_start(out=cw_sb[:], in_=cw[P * lb : P * (lb + 1), :])
                nc.vector.tensor_mul(dscratch[:], e_sb[:], cw_sb[:])
                nc.vector.tensor_reduce(
                    dcol[:], dscratch[:], axis=mybir.AxisListType.X, op=add
                )
                # + clf_bias
                cb_sb = cwpool.tile([P, 1], f32, tag="cb")
                nc.sync.dma_start(out=cb_sb[:], in_=cb[P * lb : P * (lb + 1), :])
                nc.vector.tensor_add(
                    bias_cols[:, lb : lb + 1], dcol[:], cb_sb[:]
                )

            # ---- stage 3: out.T[l, b] = W_c @ bert.T + bias_cols (fp16) ----
            for bq in (range(NB4) if "3" in STAGES else []):
                bt_sb = bpool.tile([P, KH, 512], f16, tag="bt")
                for k in range(KH):
                    nc.sync.dma_start(
                        out=bt_sb[:, k, :],
                        in_=brt[P * k : P * (k + 1), 512 * bq : 512 * (bq + 1)],
                    )
                for lb in range(NLB):
                    ps = pso.tile([P, 512], f32, tag="pso")
                    for k in range(KH):
                        nc.tensor.matmul(
                            ps[:],
                            cwt_sb[:, lb, k, :],
                            bt_sb[:, k, :],
                            start=(k == 0),
                            stop=(k == KH - 1),
                        )
                    o_sb = opool.tile([P, 512], f32, tag="o")
                    nc.scalar.add(o_sb[:], ps[:], add=bias_cols[:, lb : lb + 1])
                    nc.sync.dma_start(
                        out=out[P * lb : P * (lb + 1), 512 * bq : 512 * (bq + 1)],
                        in_=o_sb[:],
                    )

            if "3" not in STAGES:
                z = opool.tile([P, 512], f32, tag="o", name="z0")
                nc.vector.memset(z[:], 0.0)
                for lb in range(NLB):
                    for bq in range(NB4):
                        nc.sync.dma_start(
                            out=out[P * lb : P * (lb + 1), 512 * bq : 512 * (bq + 1)],
                            in_=z[:],
                        )

    nc.compile()
    return nc


def _prep_inputs(bert_cls, label_features, edges, gc_weight, gc_bias, clf_weight, clf_bias):
    """Host-side shard/layout/cast prep. Layout + dtype only — no math."""
    bf16 = ml_dtypes.bfloat16
    # lf_tiles[k, b, i, j] = LF.T[k*128+i, b*128+j] = LF[b*128+j, k*128+i]
    lf_tiles = np.ascontiguousarray(
        label_features.reshape(NLPB, P, 8, P).transpose(2, 0, 3, 1).astype(bf16)
    )
    gcw = np.ascontiguousarray(gc_weight.astype(bf16))
    gcb_row = np.ascontiguousarray(gc_bias.reshape(1, H).astype(bf16))
    bert_t = np.ascontiguousarray(bert_cls.T.astype(np.float16))

    shared = dict(lf_tiles=lf_tiles, gcw=gcw, gcb_row=gcb_row, bert_t=bert_t)
    in_maps = []
    for c in range(NCORES):
        sl = slice(c * LS, (c + 1) * LS)
        e_c = edges[sl, :]  # [1024, 8192]
        # edges_tiles[lb, k, i, j] = e_c.T[k*128+i, lb*128+j] = e_c[lb*128+j, k*128+i]
        edges_tiles = np.ascontiguousarray(
            e_c.reshape(8, P, 64, P).transpose(0, 2, 3, 1).astype(bf16)
        )
        w_c = clf_weight[sl, :]  # [1024, 1024]
        # clfwt_tiles[lb, k, i, j] = w_c.T[k*128+i, lb*128+j] = w_c[lb*128+j, k*128+i]
        clfwt_tiles = np.ascontiguousarray(
            w_c.reshape(8, P, 8, P).transpose(0, 2, 3, 1).astype(np.float16)
        )
        in_maps.append(
            dict(
                shared,
                edges_tiles=edges_tiles,
                clfwt_tiles=clfwt_tiles,
                clfw=np.ascontiguousarray(w_c),
                clfb_col=np.ascontiguousarray(clf_bias[sl].reshape(LS, 1)),
            )
        )
    return in_maps


NLPB = L // P  # 64


def kernel(**inputs):
    global LAST_RESULT
    from concourse.bass_utils import run_bass_kernel_spmd

    inputs = {k: np.asarray(v) for k, v in inputs.items()}
    nc = build_kernel()
    in_maps = _prep_inputs(**inputs)
    res = run_bass_kernel_spmd(nc, in_maps, core_ids=list(range(NCORES)))
    LAST_RESULT = res
    out_t = np.concatenate([res.results[c]["out_t"] for c in range(NCORES)], axis=0)
    return np.ascontiguousarray(out_t.T)


if __name__ == "__main__":
    rng = np.random.default_rng(0)
    ins = dict(
        bert_cls=rng.standard_normal((B, H), dtype=np.float32),
        label_features=rng.standard_normal((L, F), dtype=np.float32),
        edges=(rng.random((L, L), dtype=np.float32) / L),
        gc_weight=rng.standard_normal((F, H), dtype=np.float32) / np.sqrt(F),
        gc_bias=np.zeros(H, np.float32),
        clf_weight=rng.standard_normal((L, H), dtype=np.float32) / np.sqrt(H),
        clf_bias=np.zeros(L, np.float32),
    )
    got = kernel(**ins)
    X = ins["label_features"] @ ins["gc_weight"]
    E = np.maximum(ins["edges"] @ X + ins["gc_bias"], 0)
    diag = (E * ins["clf_weight"]).sum(1)
    exp = ins["bert_cls"] @ ins["clf_weight"].T + diag[None, :] + ins["clf_bias"][None, :]
    rel = np.linalg.norm(got - exp) / np.linalg.norm(exp)
    print("rel err:", rel)


# revision 7
# speedup vs baseline: 1.5006x; 1.5006x over previous
"""BertGCN fused kernel for 8x TRN2 NeuronCores.

Math (reference):
    X = label_features @ gc_weight                      # [L, H]
    E = relu(edges @ X + gc_bias)                       # [L, H]
    diag = sum(E * clf_weight, axis=1)                  # [L]
    out = bert_cls @ clf_weight.T + diag[None] + clf_bias[None]   # [B, L]

Sharding: label dim L split over 8 cores (1024 labels each). Each core:
  stage 1: X = LF @ GCW (full, bf16)               [8192, 1024]
  stage 2: E_c = relu(edges_c @ X + gc_bias)       [1024, 1024]
  diag_c  = rowsum(E_c * W_c)                      [1024]
  stage 3: out_c.T = W_c @ bert.T + diag_c + bias  [1024, 2048]  (fp16 matmul)
Host pre-transposes/tiles/casts operands (layout only, no FLOPs) and
re-assembles out = vstack(out_c.T).T.

All weight streams are laid out host-side as per-partition-contiguous slabs
so each DMA is one large 2D copy (128 x contiguous-bytes).

B, H, L, F = 2048, 1024, 8192, 1024.
"""

import numpy as np
import ml_dtypes

B, H, L, F = 2048, 1024, 8192, 1024
NCORES = 8
LS = L // NCORES  # 1024 labels per core
P = 128

LAST_RESULT = None


def build_kernel():
    import concourse.bass as bass  # noqa: F401
    from concourse import bacc
    import concourse.mybir as mybir
    import concourse.tile as tile

    dt = mybir.dt
    f32, bf16, f16 = dt.float32, dt.bfloat16, dt.float16
    Relu = mybir.ActivationFunctionType.Relu
    add = mybir.AluOpType.add

    nc = bacc.Bacc(None, target_bir_lowering=False, debug=False)

    # inputs (per core) — slab layouts are per-partition contiguous
    lf = nc.declare_dram_parameter("lf_slabs", [64, P, F], bf16, isOutput=False)
    gcw = nc.declare_dram_parameter("gcw_slab", [P, 8, H], bf16, isOutput=False)
    gcb = nc.declare_dram_parameter("gcb_row", [1, H], bf16, isOutput=False)
    edg = nc.declare_dram_parameter("edges_slabs", [8, P, L], bf16, isOutput=False)
    cwt = nc.declare_dram_parameter("clfwt_slab", [P, 8, 8, P], f16, isOutput=False)
    brt = nc.declare_dram_parameter("bert_t", [H, B], f16, isOutput=False)
    cw = nc.declare_dram_parameter("clfw", [LS, H], bf16, isOutput=False)
    cb = nc.declare_dram_parameter("clfb_col", [LS, 1], f32, isOutput=False)
    out = nc.declare_dram_parameter("out_t", [LS, B], f32, isOutput=True)

    KX = F // P      # 8   k-chunks for stage 1 (over F)
    NLP = L // P     # 64  l'-blocks (stage-1 M / stage-2 K)
    KL = L // P      # 64  stage-2 k-chunks (over l')
    NLB = LS // P    # 8   l-blocks of this core's label shard
    NH2 = H // 512   # 2   h-halves
    NB4 = B // 512   # 4   b-quarters (stage 3 N)
    KH = H // P      # 8   stage-3 k-chunks (over H)

    with tile.TileContext(nc) as tc:
        with (
            tc.tile_pool(name="const", bufs=1) as constp,
            tc.tile_pool(name="xk", bufs=NLP) as xpool,
            tc.tile_pool(name="big", bufs=2) as bigp,
            tc.tile_pool(name="lfslab", bufs=2) as lfp,
            tc.tile_pool(name="bstream", bufs=1) as bpool,
            tc.tile_pool(name="estream", bufs=2) as epool,
            tc.tile_pool(name="cwstream", bufs=2) as cwpool,
            tc.tile_pool(name="opool", bufs=2) as opool,
            tc.tile_pool(name="psx", bufs=2, space="PSUM") as psx,
            tc.tile_pool(name="pse", bufs=2, space="PSUM") as pse,
            tc.tile_pool(name="pso", bufs=4, space="PSUM") as pso,
        ):
            # ---- resident constants ----
            gcw_sb = bigp.tile([P, KX, H], bf16, tag="big", name="gcw_sb")
            nc.sync.dma_start(out=gcw_sb[:], in_=gcw[:])
            gcb_sb = constp.tile([1, H], bf16, tag="gcb")
            nc.sync.dma_start(out=gcb_sb[:], in_=gcb[:])
            ones1 = constp.tile([1, P], bf16, tag="ones1")
            nc.vector.memset(ones1[:], 1.0)
            cwt_sb = constp.tile([P, NLB, KH, P], f16, tag="cwt")  # [h_in, lb, h_chunk, l_in]
            nc.sync.dma_start(out=cwt_sb[:], in_=cwt[:])
            bias_cols = constp.tile([P, NLB], f32, tag="bias")  # diag+clf_bias per l-block
            dscratch = constp.tile([P, H], bf16, tag="dscratch")
            dcol = constp.tile([P, 1], f32, tag="dcol")

            # ---- stage 1: X[l', h] = LF @ GCW  (bf16) ----
            x_sb = []
            for j in range(NLP):
                x_sb.append(xpool.tile([P, H], bf16, tag="xk", name=f"x{j}"))
            for j in range(NLP):
                w = lfp.tile([P, KX, P], bf16, tag="lfw", name=f"lfw{j}")
                nc.sync.dma_start(out=w[:], in_=lf[j])
                ps = [psx.tile([P, 512], f32, tag="psx", name=f"psx{j}_{h}") for h in range(NH2)]
                for k in range(KX):
                    for h in range(NH2):
                        nc.tensor.matmul(
                            ps[h][:],
                            w[:, k, :],
                            gcw_sb[:, k, 512 * h : 512 * (h + 1)],
                            start=(k == 0),
                            stop=(k == KX - 1),
                        )
                for h in range(NH2):
                    nc.scalar.copy(x_sb[j][:, 512 * h : 512 * (h + 1)], ps[h][:])

            # ---- stage 2: E = relu(edges_c @ X + gc_bias); diag ----
            for lb in range(NLB):
                eslab = bigp.tile([P, KL, P], bf16, tag="big", name=f"eslab{lb}")
                nc.sync.dma_start(out=eslab[:], in_=edg[lb])
                e_sb = epool.tile([P, H], bf16, tag="e")
                ps = [pse.tile([P, 512], f32, tag="pse", name=f"pse{lb}_{h}") for h in range(NH2)]
                for k in range(KL):
                    for h in range(NH2):
                        nc.tensor.matmul(
                            ps[h][:],
                            eslab[:, k, :],
                            x_sb[k][:, 512 * h : 512 * (h + 1)],
                            start=(k == 0),
                            stop=False,
                        )
                for h in range(NH2):
                    # + gc_bias via K=1 accumulation row
                    nc.tensor.matmul(
                        ps[h][:],
                        ones1[:],
                        gcb_sb[:, 512 * h : 512 * (h + 1)],
                        start=False,
                        stop=True,
                    )
                    nc.scalar.activation(
                        e_sb[:, 512 * h : 512 * (h + 1)], ps[h][:], Relu
                    )
                # diag_c[lb] = rowsum(E_lb * W_lb) on DVE
                cw_sb = cwpool.tile([P, H], bf16, tag="cw")
                nc.sync.dma_start(out=cw_sb[:], in_=cw[P * lb : P * (lb + 1), :])
                nc.vector.tensor_mul(dscratch[:], e_sb[:], cw_sb[:])
                nc.vector.tensor_reduce(
                    dcol[:], dscratch[:], axis=mybir.AxisListType.X, op=add
                )
                # + clf_bias
                cb_sb = cwpool.tile([P, 1], f32, tag="cb")
                nc.sync.dma_start(out=cb_sb[:], in_=cb[P * lb : P * (lb + 1), :])
                nc.vector.tensor_add(
                    bias_cols[:, lb : lb + 1], dcol[:], cb_sb[:]
                )

            # ---- stage 3: out.T[l, b] = W_c @ bert.T + bias_cols (fp16) ----
            brt_r = brt.rearrange("(k p) b -> p k b", p=P)
            for bq in range(NB4):
                bt_sb = bpool.tile([P, KH, 512], f16, tag="bt")
                nc.sync.dma_start(
                    out=bt_sb[:], in_=brt_r[:, :, 512 * bq : 512 * (bq + 1)]
                )
                for lb in range(NLB):
                    ps = pso.tile([P, 512], f32, tag="pso")
                    for k in range(KH):
                        nc.tensor.matmul(
                            ps[:],
                            cwt_sb[:, lb, k, :],
                            bt_sb[:, k, :],
                            start=(k == 0),
                            stop=(k == KH - 1),
                        )
                    o_sb = opool.tile([P, 512], f32, tag="o")
                    nc.scalar.add(o_sb[:], ps[:], add=bias_cols[:, lb : lb + 1])
                    nc.sync.dma_start(
                        out=out[P * lb : P * (lb + 1), 512 * bq : 512 * (bq + 1)],
                        in_=o_sb[:],
                    )

    nc.compile()
    return nc


def _prep_inputs(bert_cls, label_features, edges, gc_weight, gc_bias, clf_weight, clf_bias):
    """Host-side shard/layout/cast prep. Layout + dtype only — no math."""
    bf16 = ml_dtypes.bfloat16
    # lf_slabs[b, i, k*128+j] = LF.T[k*128+i, b*128+j] = LF[b*128+j, k*128+i]
    lf_slabs = np.ascontiguousarray(
        label_features.reshape(64, P, 8, P).transpose(0, 3, 2, 1).astype(bf16).reshape(64, P, F)
    )
    # gcw_slab[i, k, h] = gc_weight[k*128+i, h]
    gcw_slab = np.ascontiguousarray(
        gc_weight.reshape(8, P, H).transpose(1, 0, 2).astype(bf16)
    )
    gcb_row = np.ascontiguousarray(gc_bias.reshape(1, H).astype(bf16))
    bert_t = np.ascontiguousarray(bert_cls.T.astype(np.float16))

    shared = dict(lf_slabs=lf_slabs, gcw_slab=gcw_slab, gcb_row=gcb_row, bert_t=bert_t)
    in_maps = []
    for c in range(NCORES):
        sl = slice(c * LS, (c + 1) * LS)
        e_c = edges[sl, :]  # [1024, 8192]
        # edges_slabs[lb, i, k*128+j] = e_c[lb*128+j, k*128+i]
        edges_slabs = np.ascontiguousarray(
            e_c.reshape(8, P, 64, P).transpose(0, 3, 2, 1).astype(bf16).reshape(8, P, L)
        )
        w_c = clf_weight[sl, :]  # [1024, 1024]
        # clfwt_slab[i, lb, k, j] = w_c[lb*128+j, k*128+i]
        clfwt_slab = np.ascontiguousarray(
            w_c.reshape(8, P, 8, P).transpose(3, 0, 2, 1).astype(np.float16)
        )
        in_maps.append(
            dict(
                shared,
                edges_slabs=edges_slabs,
                clfwt_slab=clfwt_slab,
                clfw=np.ascontiguousarray(w_c.astype(bf16)),
                clfb_col=np.ascontiguousarray(clf_bias[sl].reshape(LS, 1)),
            )
        )
    return in_maps


def kernel(**inputs):
    global LAST_RESULT
    from concourse.bass_utils import run_bass_kernel_spmd

    inputs = {k: np.asarray(v) for k, v in inputs.items()}
    nc = build_kernel()
    in_maps = _prep_inputs(**inputs)
    res = run_bass_kernel_spmd(nc, in_maps, core_ids=list(range(NCORES)))
    LAST_RESULT = res
    out_t = np.concatenate([res.results[c]["out_t"] for c in range(NCORES)], axis=0)
    return np.ascontiguousarray(out_t.T)


if __name__ == "__main__":
    rng = np.random.default_rng(0)
    ins = dict(
        bert_cls=rng.standard_normal((B, H), dtype=np.float32),
        label_features=rng.standard_normal((L, F), dtype=np.float32),
        edges=(rng.random((L, L), dtype=np.float32) / L),
        gc_weight=rng.standard_normal((F, H), dtype=np.float32) / np.sqrt(F),
        gc_bias=np.zeros(H, np.float32),
        clf_weight=rng.standard_normal((L, H), dtype=np.float32) / np.sqrt(H),
        clf_bias=np.zeros(L, np.float32),
    )
    got = kernel(**ins)
    X = ins["label_features"] @ ins["gc_weight"]
    E = np.maximum(ins["edges"] @ X + ins["gc_bias"], 0)
    diag = (E * ins["clf_weight"]).sum(1)
    exp = ins["bert_cls"] @ ins["clf_weight"].T + diag[None, :] + ins["clf_bias"][None, :]
    rel = np.linalg.norm(got - exp) / np.linalg.norm(exp)
    print("rel err:", rel)


# revision 9
# speedup vs baseline: 2.0546x; 1.3692x over previous
"""BertGCN fused kernel for 8x TRN2 NeuronCores.

Math (reference):
    X = label_features @ gc_weight                      # [L, H]
    E = relu(edges @ X + gc_bias)                       # [L, H]
    diag = sum(E * clf_weight, axis=1)                  # [L]
    out = bert_cls @ clf_weight.T + diag[None] + clf_bias[None]   # [B, L]

Two SPMD launches over 8 cores (label dim L sharded, 1024 labels/core):
  launch 1: X row-shard per core: X[c*1024:(c+1)*1024] = LF_c @ GCW (bf16).
            Host gathers the 8 shards and rebroadcasts full X (layout only).
  launch 2: E_c = relu(edges_c @ X + gc_bias); diag_c = rowsum(E_c * W_c);
            out_c.T = W_c @ bert.T + diag_c + clf_bias   (fp16 logits matmul)
Host pre-transposes/tiles/casts operands (layout only, no FLOPs) and
re-assembles out = vstack(out_c.T).T.

All weight streams are laid out host-side as per-partition-contiguous slabs
so each DMA is one large 2D copy (128 x contiguous-bytes).

B, H, L, F = 2048, 1024, 8192, 1024.
"""

import numpy as np
import ml_dtypes

B, H, L, F = 2048, 1024, 8192, 1024
NCORES = 8
LS = L // NCORES  # 1024 labels per core
P = 128

LAST_RESULTS = []


def _mybir():
    import concourse.mybir as mybir

    return mybir


def build_kernel_x():
    """Launch 1: per-core X row-shard = LF_c @ GCW."""
    from concourse import bacc
    import concourse.mybir as mybir
    import concourse.tile as tile

    dt = mybir.dt
    bf16 = dt.bfloat16
    f32 = dt.float32

    nc = bacc.Bacc(None, target_bir_lowering=False, debug=False)
    lf = nc.declare_dram_parameter("lf_slabs", [8, P, F], bf16, isOutput=False)
    gcw = nc.declare_dram_parameter("gcw_slab", [P, 8, H], bf16, isOutput=False)
    xout = nc.declare_dram_parameter("x_slabs", [8, P, H], bf16, isOutput=True)

    KX = F // P
    NH2 = H // 512

    with tile.TileContext(nc) as tc:
        with (
            tc.tile_pool(name="const", bufs=1) as constp,
            tc.tile_pool(name="lfslab", bufs=3) as lfp,
            tc.tile_pool(name="xo", bufs=3) as xop,
            tc.tile_pool(name="psx", bufs=4, space="PSUM") as psx,
        ):
            gcw_sb = constp.tile([P, KX, H], bf16, tag="gcw")
            nc.sync.dma_start(out=gcw_sb[:], in_=gcw[:])
            for j in range(8):
                w = lfp.tile([P, KX, P], bf16, tag="lfw", name=f"lfw{j}")
                nc.sync.dma_start(out=w[:], in_=lf[j])
                ps = [psx.tile([P, 512], f32, tag="psx", name=f"psx{j}_{h}") for h in range(NH2)]
                for k in range(KX):
                    for h in range(NH2):
                        nc.tensor.matmul(
                            ps[h][:],
                            w[:, k, :],
                            gcw_sb[:, k, 512 * h : 512 * (h + 1)],
                            start=(k == 0),
                            stop=(k == KX - 1),
                        )
                xo = xop.tile([P, H], bf16, tag="xo", name=f"xo{j}")
                for h in range(NH2):
                    nc.scalar.copy(xo[:, 512 * h : 512 * (h + 1)], ps[h][:])
                nc.sync.dma_start(out=xout[j], in_=xo[:])

    nc.compile()
    return nc


def build_kernel_main():
    """Launch 2: E, diag, logits, output (per core label shard)."""
    from concourse import bacc
    import concourse.mybir as mybir
    import concourse.tile as tile

    dt = mybir.dt
    f32, bf16, f16 = dt.float32, dt.bfloat16, dt.float16
    add = mybir.AluOpType.add
    amax = mybir.AluOpType.max
    mult = mybir.AluOpType.mult

    nc = bacc.Bacc(None, target_bir_lowering=False, debug=False)

    xin = nc.declare_dram_parameter("x_slabs", [64, P, H], bf16, isOutput=False)
    gcb = nc.declare_dram_parameter("gcb_row", [1, H], bf16, isOutput=False)
    edg = nc.declare_dram_parameter("edges_slabs", [8, P, L], bf16, isOutput=False)
    cwt = nc.declare_dram_parameter("clfwt_slab", [P, 8, 8, P], f16, isOutput=False)
    brt = nc.declare_dram_parameter("bert_t", [H, B], f16, isOutput=False)
    cw = nc.declare_dram_parameter("clfw", [LS, H], bf16, isOutput=False)
    cb = nc.declare_dram_parameter("clfb_col", [LS, 1], f32, isOutput=False)
    out = nc.declare_dram_parameter("out_t", [LS, B], f32, isOutput=True)

    NLP = L // P     # 64  l'-chunks (stage-2 K)
    KL = L // P      # 64
    NLB = LS // P    # 8   l-blocks of this core's label shard
    NH2 = H // 512   # 2   h-halves
    NB4 = B // 512   # 4   b-quarters (stage 3 N)
    KH = H // P      # 8   stage-3 k-chunks (over H)

    with tile.TileContext(nc) as tc:
        with (
            tc.tile_pool(name="const", bufs=1) as constp,
            tc.tile_pool(name="xk", bufs=NLP) as xpool,
            tc.tile_pool(name="eslab", bufs=2) as esp,
            tc.tile_pool(name="bstream", bufs=2) as bpool,
            tc.tile_pool(name="cwstream", bufs=2) as cwpool,
            tc.tile_pool(name="opool", bufs=2) as opool,
            tc.tile_pool(name="pse", bufs=2, space="PSUM") as pse,
            tc.tile_pool(name="pso", bufs=4, space="PSUM") as pso,
        ):
            # ---- resident constants ----
            gcb_sb = constp.tile([1, H], bf16, tag="gcb")
            nc.sync.dma_start(out=gcb_sb[:], in_=gcb[:])
            ones1 = constp.tile([1, P], bf16, tag="ones1")
            nc.vector.memset(ones1[:], 1.0)
            cwt_sb = constp.tile([P, NLB, KH, P], f16, tag="cwt")
            nc.sync.dma_start(out=cwt_sb[:], in_=cwt[:])
            dscratch = constp.tile([P, H], bf16, tag="dscratch")
            dcol = constp.tile([P, 1], f32, tag="dcol")
            bias_col = [
                constp.tile([P, 1], f32, tag=f"bias{lb}", name=f"bias{lb}")
                for lb in range(NLB)
            ]

            # ---- load X (computed in launch 1) ----
            x_sb = []
            for j in range(NLP):
                x_sb.append(xpool.tile([P, H], bf16, tag="xk", name=f"x{j}"))
            for j in range(NLP):
                nc.sync.dma_start(out=x_sb[j][:], in_=xin[j])

            # ---- stage 2: E = relu(edges_c @ X + gc_bias); diag ----
            for lb in range(NLB):
                eslab = esp.tile([P, KL, P], bf16, tag="eslab", name=f"eslab{lb}")
                nc.sync.dma_start(out=eslab[:], in_=edg[lb])
                cw_sb = cwpool.tile([P, H], bf16, tag="cw", name=f"cw{lb}")
                nc.sync.dma_start(out=cw_sb[:], in_=cw[P * lb : P * (lb + 1), :])
                ps = [pse.tile([P, 512], f32, tag="pse", name=f"pse{lb}_{h}") for h in range(NH2)]
                for k in range(KL):
                    for h in range(NH2):
                        nc.tensor.matmul(
                            ps[h][:],
                            eslab[:, k, :],
                            x_sb[k][:, 512 * h : 512 * (h + 1)],
                            start=(k == 0),
                            stop=False,
                        )
                for h in range(NH2):
                    # + gc_bias via K=1 accumulation row
                    nc.tensor.matmul(
                        ps[h][:],
                        ones1[:],
                        gcb_sb[:, 512 * h : 512 * (h + 1)],
                        start=False,
                        stop=True,
                    )
                    # fused relu(E)*W product straight out of PSUM
                    nc.vector.scalar_tensor_tensor(
                        dscratch[:, 512 * h : 512 * (h + 1)],
                        ps[h][:],
                        0.0,
                        cw_sb[:, 512 * h : 512 * (h + 1)],
                        op0=amax,
                        op1=mult,
                    )
                nc.vector.tensor_reduce(
                    dcol[:], dscratch[:], axis=mybir.AxisListType.X, op=add
                )
                # + clf_bias
                cb_sb = cwpool.tile([P, 1], f32, tag="cb")
                nc.sync.dma_start(out=cb_sb[:], in_=cb[P * lb : P * (lb + 1), :])
                nc.vector.tensor_add(bias_col[lb][:], dcol[:], cb_sb[:])

            # ---- stage 3: out.T[l, b] = W_c @ bert.T + bias (fp16) ----
            brt_r = brt.rearrange("(k p) b -> p k b", p=P)
            for bq in range(NB4):
                bt_sb = bpool.tile([P, KH, 512], f16, tag="bt", name=f"bt{bq}")
                nc.sync.dma_start(
                    out=bt_sb[:], in_=brt_r[:, :, 512 * bq : 512 * (bq + 1)]
                )
                for lb in range(NLB):
                    ps = pso.tile([P, 512], f32, tag="pso")
                    for k in range(KH):
                        nc.tensor.matmul(
                            ps[:],
                            cwt_sb[:, lb, k, :],
                            bt_sb[:, k, :],
                            start=(k == 0),
                            stop=(k == KH - 1),
                        )
                    o_sb = opool.tile([P, 512], f32, tag="o")
                    nc.scalar.add(o_sb[:], ps[:], add=bias_col[lb][:])
                    nc.sync.dma_start(
                        out=out[P * lb : P * (lb + 1), 512 * bq : 512 * (bq + 1)],
                        in_=o_sb[:],
                    )

    nc.compile()
    return nc


def _prep_inputs(bert_cls, label_features, edges, gc_weight, gc_bias, clf_weight, clf_bias):
    """Host-side shard/layout/cast prep. Layout + dtype only — no math."""
    bf16 = ml_dtypes.bfloat16
    # lf_slabs[b, i, k*128+j] = LF[b*128+j, k*128+i]  (b = l'-block)
    lf_slabs = np.ascontiguousarray(
        label_features.reshape(64, P, 8, P).transpose(0, 3, 2, 1).astype(bf16).reshape(64, P, F)
    )
    # gcw_slab[i, k, h] = gc_weight[k*128+i, h]
    gcw_slab = np.ascontiguousarray(
        gc_weight.reshape(8, P, H).transpose(1, 0, 2).astype(bf16)
    )
    gcb_row = np.ascontiguousarray(gc_bias.reshape(1, H).astype(bf16))
    bert_t = np.ascontiguousarray(bert_cls.T.astype(np.float16))

    x_maps = [
        dict(lf_slabs=lf_slabs[c * 8 : (c + 1) * 8], gcw_slab=gcw_slab)
        for c in range(NCORES)
    ]

    shared = dict(gcb_row=gcb_row, bert_t=bert_t)
    main_maps = []
    for c in range(NCORES):
        sl = slice(c * LS, (c + 1) * LS)
        e_c = edges[sl, :]  # [1024, 8192]
        # edges_slabs[lb, i, k*128+j] = e_c[lb*128+j, k*128+i]
        edges_slabs = np.ascontiguousarray(
            e_c.reshape(8, P, 64, P).transpose(0, 3, 2, 1).astype(bf16).reshape(8, P, L)
        )
        w_c = clf_weight[sl, :]  # [1024, 1024]
        # clfwt_slab[i, lb, k, j] = w_c[lb*128+j, k*128+i]
        clfwt_slab = np.ascontiguousarray(
            w_c.reshape(8, P, 8, P).transpose(3, 0, 2, 1).astype(np.float16)
        )
        main_maps.append(
            dict(
                shared,
                edges_slabs=edges_slabs,
                clfwt_slab=clfwt_slab,
                clfw=np.ascontiguousarray(w_c.astype(bf16)),
                clfb_col=np.ascontiguousarray(clf_bias[sl].reshape(LS, 1)),
            )
        )
    return x_maps, main_maps


def kernel(**inputs):
    global LAST_RESULTS
    from concourse.bass_utils import run_bass_kernel_spmd

    inputs = {k: np.asarray(v) for k, v in inputs.items()}
    x_maps, main_maps = _prep_inputs(**inputs)

    nc_x = build_kernel_x()
    res_x = run_bass_kernel_spmd(nc_x, x_maps, core_ids=list(range(NCORES)))
    # gather X shards -> full X in stage-2 rhs slab layout [64, P, H]
    x_full = np.concatenate(
        [res_x.results[c]["x_slabs"] for c in range(NCORES)], axis=0
    )
    for m in main_maps:
        m["x_slabs"] = x_full

    nc_main = build_kernel_main()
    res = run_bass_kernel_spmd(nc_main, main_maps, core_ids=list(range(NCORES)))
    LAST_RESULTS = [res_x, res]
    out_t = np.concatenate([res.results[c]["out_t"] for c in range(NCORES)], axis=0)
    return np.ascontiguousarray(out_t.T)


if __name__ == "__main__":
    rng = np.random.default_rng(0)
    ins = dict(
        bert_cls=rng.standard_normal((B, H), dtype=np.float32),
        label_features=rng.standard_normal((L, F), dtype=np.float32),
        edges=(rng.random((L, L), dtype=np.float32) / L),
        gc_weight=rng.standard_normal((F, H), dtype=np.float32) / np.sqrt(F),
        gc_bias=np.zeros(H, np.float32),
        clf_weight=rng.standard_normal((L, H), dtype=np.float32) / np.sqrt(H),
        clf_bias=np.zeros(L, np.float32),
    )
    got = kernel(**ins)
    X = ins["label_features"] @ ins["gc_weight"]
    E = np.maximum(ins["edges"] @ X + ins["gc_bias"], 0)
    diag = (E * ins["clf_weight"]).sum(1)
    exp = ins["bert_cls"] @ ins["clf_weight"].T + diag[None, :] + ins["clf_bias"][None, :]
    rel = np.linalg.norm(got - exp) / np.linalg.norm(exp)
    print("rel err:", rel)


# revision 10
# speedup vs baseline: 2.2715x; 1.1055x over previous
"""BertGCN fused kernel for 8x TRN2 NeuronCores.

Math (reference):
    X = label_features @ gc_weight                      # [L, H]
    E = relu(edges @ X + gc_bias)                       # [L, H]
    diag = sum(E * clf_weight, axis=1)                  # [L]
    out = bert_cls @ clf_weight.T + diag[None] + clf_bias[None]   # [B, L]

Two SPMD launches over 8 cores (label dim L sharded, 1024 labels/core):
  launch 1: X row-shard per core: X[c*1024:(c+1)*1024] = LF_c @ GCW (bf16).
            Host gathers the 8 shards and rebroadcasts full X (layout only).
  launch 2: E_c = relu(edges_c @ X + gc_bias); diag_c = rowsum(E_c * W_c);
            out_c.T = W_c @ bert.T + diag_c + clf_bias   (fp16 logits matmul)
Host pre-transposes/tiles/casts operands (layout only, no FLOPs) and
re-assembles out = vstack(out_c.T).T.

All weight streams are laid out host-side as per-partition-contiguous slabs
so each DMA is one large 2D copy (128 x contiguous-bytes).

B, H, L, F = 2048, 1024, 8192, 1024.
"""

import numpy as np
import ml_dtypes

B, H, L, F = 2048, 1024, 8192, 1024
NCORES = 8
LS = L // NCORES  # 1024 labels per core
P = 128

LAST_RESULTS = []


def _mybir():
    import concourse.mybir as mybir

    return mybir


def build_kernel_x():
    """Launch 1: per-core X row-shard = LF_c @ GCW."""
    from concourse import bacc
    import concourse.mybir as mybir
    import concourse.tile as tile

    dt = mybir.dt
    bf16 = dt.bfloat16
    f32 = dt.float32

    nc = bacc.Bacc(None, target_bir_lowering=False, debug=False)
    lf = nc.declare_dram_parameter("lf_slabs", [8, P, F], bf16, isOutput=False)
    gcw = nc.declare_dram_parameter("gcw_slab", [P, 8, H], bf16, isOutput=False)
    xout = nc.declare_dram_parameter("x_slabs", [8, P, H], bf16, isOutput=True)

    KX = F // P
    NH2 = H // 512

    with tile.TileContext(nc) as tc:
        with (
            tc.tile_pool(name="const", bufs=1) as constp,
            tc.tile_pool(name="lfslab", bufs=3) as lfp,
            tc.tile_pool(name="xo", bufs=3) as xop,
            tc.tile_pool(name="psx", bufs=4, space="PSUM") as psx,
        ):
            w0 = lfp.tile([P, KX, P], bf16, tag="lfw", name="lfw0")
            nc.sync.dma_start(out=w0[:], in_=lf[0])
            gcw_sb = constp.tile([P, KX, H], bf16, tag="gcw")
            for k in range(KX):
                nc.sync.dma_start(out=gcw_sb[:, k, :], in_=gcw[:, k, :])
            for j in range(8):
                if j == 0:
                    w = w0
                else:
                    w = lfp.tile([P, KX, P], bf16, tag="lfw", name=f"lfw{j}")
                    nc.sync.dma_start(out=w[:], in_=lf[j])
                ps = [psx.tile([P, 512], f32, tag="psx", name=f"psx{j}_{h}") for h in range(NH2)]
                for k in range(KX):
                    for h in range(NH2):
                        nc.tensor.matmul(
                            ps[h][:],
                            w[:, k, :],
                            gcw_sb[:, k, 512 * h : 512 * (h + 1)],
                            start=(k == 0),
                            stop=(k == KX - 1),
                        )
                xo = xop.tile([P, H], bf16, tag="xo", name=f"xo{j}")
                for h in range(NH2):
                    nc.scalar.copy(xo[:, 512 * h : 512 * (h + 1)], ps[h][:])
                nc.sync.dma_start(out=xout[j], in_=xo[:])

    nc.compile()
    return nc


def build_kernel_main():
    """Launch 2: E, diag, logits, output (per core label shard)."""
    from concourse import bacc
    import concourse.mybir as mybir
    import concourse.tile as tile

    dt = mybir.dt
    f32, bf16, f16 = dt.float32, dt.bfloat16, dt.float16
    add = mybir.AluOpType.add
    amax = mybir.AluOpType.max
    mult = mybir.AluOpType.mult

    nc = bacc.Bacc(None, target_bir_lowering=False, debug=False)

    xin = nc.declare_dram_parameter("x_slabs", [64, P, H], bf16, isOutput=False)
    gcb = nc.declare_dram_parameter("gcb_row", [1, H], bf16, isOutput=False)
    edg = nc.declare_dram_parameter("edges_slabs", [8, P, L], bf16, isOutput=False)
    cwt = nc.declare_dram_parameter("clfwt_slab", [P, 8, 8, P], f16, isOutput=False)
    brt = nc.declare_dram_parameter("bert_t", [H, B], f16, isOutput=False)
    cw = nc.declare_dram_parameter("clfw", [LS, H], bf16, isOutput=False)
    cb = nc.declare_dram_parameter("clfb_col", [LS, 1], f32, isOutput=False)
    out = nc.declare_dram_parameter("out_t", [LS, B], f32, isOutput=True)

    NLP = L // P     # 64  l'-chunks (stage-2 K)
    KL = L // P      # 64
    NLB = LS // P    # 8   l-blocks of this core's label shard
    NH2 = H // 512   # 2   h-halves
    NB4 = B // 512   # 4   b-quarters (stage 3 N)
    KH = H // P      # 8   stage-3 k-chunks (over H)

    with tile.TileContext(nc) as tc:
        with (
            tc.tile_pool(name="const", bufs=1) as constp,
            tc.tile_pool(name="xk", bufs=NLP) as xpool,
            tc.tile_pool(name="eslab", bufs=2) as esp,
            tc.tile_pool(name="bstream", bufs=2) as bpool,
            tc.tile_pool(name="cwstream", bufs=2) as cwpool,
            tc.tile_pool(name="opool", bufs=2) as opool,
            tc.tile_pool(name="pse", bufs=2, space="PSUM") as pse,
            tc.tile_pool(name="pso", bufs=4, space="PSUM") as pso,
        ):
            # ---- resident constants ----
            gcb_sb = constp.tile([1, H], bf16, tag="gcb")
            nc.sync.dma_start(out=gcb_sb[:], in_=gcb[:])
            ones1 = constp.tile([1, P], bf16, tag="ones1")
            nc.vector.memset(ones1[:], 1.0)
            cwt_sb = constp.tile([P, NLB, KH, P], f16, tag="cwt")
            dscratch = constp.tile([P, H], bf16, tag="dscratch")
            dcol = constp.tile([P, 1], f32, tag="dcol")
            bias_col = [
                constp.tile([P, 1], f32, tag=f"bias{lb}", name=f"bias{lb}")
                for lb in range(NLB)
            ]

            # first edges slab before the X stream so the PE can start early
            eslab0 = esp.tile([P, KL, P], bf16, tag="eslab", name="eslab0")
            nc.sync.dma_start(out=eslab0[:], in_=edg[0])

            # ---- load X (computed in launch 1) ----
            x_sb = []
            for j in range(NLP):
                x_sb.append(xpool.tile([P, H], bf16, tag="xk", name=f"x{j}"))
            for j in range(NLP):
                nc.sync.dma_start(out=x_sb[j][:], in_=xin[j])

            # ---- stage 2: E = relu(edges_c @ X + gc_bias); diag ----
            for lb in range(NLB):
                if lb == 0:
                    eslab = eslab0
                else:
                    eslab = esp.tile([P, KL, P], bf16, tag="eslab", name=f"eslab{lb}")
                    nc.sync.dma_start(out=eslab[:], in_=edg[lb])
                cw_sb = cwpool.tile([P, H], bf16, tag="cw", name=f"cw{lb}")
                nc.sync.dma_start(out=cw_sb[:], in_=cw[P * lb : P * (lb + 1), :])
                ps = [pse.tile([P, 512], f32, tag="pse", name=f"pse{lb}_{h}") for h in range(NH2)]
                for k in range(KL):
                    for h in range(NH2):
                        nc.tensor.matmul(
                            ps[h][:],
                            eslab[:, k, :],
                            x_sb[k][:, 512 * h : 512 * (h + 1)],
                            start=(k == 0),
                            stop=False,
                        )
                for h in range(NH2):
                    # + gc_bias via K=1 accumulation row
                    nc.tensor.matmul(
                        ps[h][:],
                        ones1[:],
                        gcb_sb[:, 512 * h : 512 * (h + 1)],
                        start=False,
                        stop=True,
                    )
                    # fused relu(E)*W product straight out of PSUM
                    nc.vector.scalar_tensor_tensor(
                        dscratch[:, 512 * h : 512 * (h + 1)],
                        ps[h][:],
                        0.0,
                        cw_sb[:, 512 * h : 512 * (h + 1)],
                        op0=amax,
                        op1=mult,
                    )
                nc.vector.tensor_reduce(
                    dcol[:], dscratch[:], axis=mybir.AxisListType.X, op=add
                )
                # + clf_bias
                cb_sb = cwpool.tile([P, 1], f32, tag="cb")
                nc.sync.dma_start(out=cb_sb[:], in_=cb[P * lb : P * (lb + 1), :])
                nc.vector.tensor_add(bias_col[lb][:], dcol[:], cb_sb[:])

            # ---- stage 3: out.T[l, b] = W_c @ bert.T + bias (fp16) ----
            nc.sync.dma_start(out=cwt_sb[:], in_=cwt[:])
            brt_r = brt.rearrange("(k p) b -> p k b", p=P)
            for bq in range(NB4):
                bt_sb = bpool.tile([P, KH, 512], f16, tag="bt", name=f"bt{bq}")
                nc.sync.dma_start(
                    out=bt_sb[:], in_=brt_r[:, :, 512 * bq : 512 * (bq + 1)]
                )
                for lb in range(NLB):
                    ps = pso.tile([P, 512], f32, tag="pso")
                    for k in range(KH):
                        nc.tensor.matmul(
                            ps[:],
                            cwt_sb[:, lb, k, :],
                            bt_sb[:, k, :],
                            start=(k == 0),
                            stop=(k == KH - 1),
                        )
                    o_sb = opool.tile([P, 512], f32, tag="o")
                    nc.scalar.add(o_sb[:], ps[:], add=bias_col[lb][:])
                    nc.sync.dma_start(
                        out=out[P * lb : P * (lb + 1), 512 * bq : 512 * (bq + 1)],
                        in_=o_sb[:],
                    )

    nc.compile()
    return nc


def _prep_inputs(bert_cls, label_features, edges, gc_weight, gc_bias, clf_weight, clf_bias):
    """Host-side shard/layout/cast prep. Layout + dtype only — no math."""
    bf16 = ml_dtypes.bfloat16
    # lf_slabs[b, i, k*128+j] = LF[b*128+j, k*128+i]  (b = l'-block)
    lf_slabs = np.ascontiguousarray(
        label_features.reshape(64, P, 8, P).transpose(0, 3, 2, 1).astype(bf16).reshape(64, P, F)
    )
    # gcw_slab[i, k, h] = gc_weight[k*128+i, h]
    gcw_slab = np.ascontiguousarray(
        gc_weight.reshape(8, P, H).transpose(1, 0, 2).astype(bf16)
    )
    gcb_row = np.ascontiguousarray(gc_bias.reshape(1, H).astype(bf16))
    bert_t = np.ascontiguousarray(bert_cls.T.astype(np.float16))

    x_maps = [
        dict(lf_slabs=lf_slabs[c * 8 : (c + 1) * 8], gcw_slab=gcw_slab)
        for c in range(NCORES)
    ]

    shared = dict(gcb_row=gcb_row, bert_t=bert_t)
    main_maps = []
    for c in range(NCORES):
        sl = slice(c * LS, (c + 1) * LS)
        e_c = edges[sl, :]  # [1024, 8192]
        # edges_slabs[lb, i, k*128+j] = e_c[lb*128+j, k*128+i]
        edges_slabs = np.ascontiguousarray(
            e_c.reshape(8, P, 64, P).transpose(0, 3, 2, 1).astype(bf16).reshape(8, P, L)
        )
        w_c = clf_weight[sl, :]  # [1024, 1024]
        # clfwt_slab[i, lb, k, j] = w_c[lb*128+j, k*128+i]
        clfwt_slab = np.ascontiguousarray(
            w_c.reshape(8, P, 8, P).transpose(3, 0, 2, 1).astype(np.float16)
        )
        main_maps.append(
            dict(
                shared,
                edges_slabs=edges_slabs,
                clfwt_slab=clfwt_slab,
                clfw=np.ascontiguousarray(w_c.astype(bf16)),
                clfb_col=np.ascontiguousarray(clf_bias[sl].reshape(LS, 1)),
            )
        )
    return x_maps, main_maps


def kernel(**inputs):
    global LAST_RESULTS
    from concourse.bass_utils import run_bass_kernel_spmd

    inputs = {k: np.asarray(v) for k, v in inputs.items()}
    x_maps, main_maps = _prep_inputs(**inputs)

    nc_x = build_kernel_x()
    res_x = run_bass_kernel_spmd(nc_x, x_maps, core_ids=list(range(NCORES)))
    # gather X shards -> full X in stage-2 rhs slab layout [64, P, H]
    x_full = np.concatenate(
        [res_x.results[c]["x_slabs"] for c in range(NCORES)], axis=0
    )
    for m in main_maps:
        m["x_slabs"] = x_full

    nc_main = build_kernel_main()
    res = run_bass_kernel_spmd(nc_main, main_maps, core_ids=list(range(NCORES)))
    LAST_RESULTS = [res_x, res]
    out_t = np.concatenate([res.results[c]["out_t"] for c in range(NCORES)], axis=0)
    return np.ascontiguousarray(out_t.T)


if __name__ == "__main__":
    rng = np.random.default_rng(0)
    ins = dict(
        bert_cls=rng.standard_normal((B, H), dtype=np.float32),
        label_features=rng.standard_normal((L, F), dtype=np.float32),
        edges=(rng.random((L, L), dtype=np.float32) / L),
        gc_weight=rng.standard_normal((F, H), dtype=np.float32) / np.sqrt(F),
        gc_bias=np.zeros(H, np.float32),
        clf_weight=rng.standard_normal((L, H), dtype=np.float32) / np.sqrt(H),
        clf_bias=np.zeros(L, np.float32),
    )
    got = kernel(**ins)
    X = ins["label_features"] @ ins["gc_weight"]
    E = np.maximum(ins["edges"] @ X + ins["gc_bias"], 0)
    diag = (E * ins["clf_weight"]).sum(1)
    exp = ins["bert_cls"] @ ins["clf_weight"].T + diag[None, :] + ins["clf_bias"][None, :]
    rel = np.linalg.norm(got - exp) / np.linalg.norm(exp)
    print("rel err:", rel)
